# revision 4
# baseline (speedup 1.0000x reference)
"""Trainium2 Bass kernel for nn_Attention_81870666597078 (v2).

Multi-head causal self-attention (b=4, s=2048, d=1024, 16 heads) with QKV/O
projections. 2D sharding over the 8 NeuronCores: core c owns batch c//2 and
head-group c%2 (8 heads = 512 hidden dims). Each core computes its heads'
attention over its batch plus a partial O projection for the full 1024 output
dims; the host sums the 2 head-group partials per batch.

Per-core dataflow (matmuls fp16 with fp32 PSUM accumulation):
  - QKV projection of q/k into transposed [dims, seq] layout (weight
    stationary), two heads per 128-partition tile (head-pair tiles).
  - v projected directly into natural [seq, dh] layout (x stationary), with a
    fused ones column per head ([v | 1]) so the PV matmul also emits the
    softmax denominator as output row 64. No PE transposes anywhere.
  - Scores computed transposed, S^T [k, q], two heads per PSUM pair tile at
    base partitions 0/64 so the PE row groups run both concurrently. One exp
    activation covers both heads; causal masking is a multiplicative 0/1 mask
    applied on the diagonal band by the (otherwise idle) GPSIMD engine.
    Fully-masked column ranges are sliced away from matmul + exp.
  - Normalization: PV outputs are evacuated PSUM->SBUF, the denominator row
    reciprocal'd (2 rows at once), broadcast to 128 partitions via one K=2
    selector matmul, and multiplied into the aoT tiles.
  - O projection from aoT tiles (stationary) and W_o^T (moving); partial
    [seq, 1024] written out as fp16.
  - QKV matmuls of the next head-pair, the v pass, and the O projection are
    interleaved into the attention instruction stream as PE filler so the
    tensor engine stays busy while the (serial-bottleneck) exp runs on the
    scalar engine.
"""
import os
from collections import deque

import numpy as np

import concourse.bass as bass  # noqa: F401
import concourse.mybir as mybir
from concourse import bacc, library_config
from concourse.bass_utils import run_bass_kernel_spmd
from concourse.tile import TileContext

dt = mybir.dt
F32 = dt.float32
F16 = dt.float16
F32R = dt.float32r
Exp = mybir.ActivationFunctionType.Exp

N_CORES = 8
B = 4
S = 2048
D = 1024
NH = 8            # heads per core
CD = 512          # head dims per core (8 heads x 64)
NPAIR = 4         # head pairs per core
NDT = D // 128    # 8 k-tiles over the model dim
NST = S // 512    # 4 seq tiles of 512
NSC = S // 128    # 16 seq chunks of 128


def _build_bass():
    nc = bacc.Bacc("TRN2", target_bir_lowering=False, debug=False)
    xt = nc.dram_tensor("xt", [D, S], F16, kind="ExternalInput")
    wqk = nc.dram_tensor("wqk", [D, 2 * CD], F16, kind="ExternalInput")
    wv = nc.dram_tensor("wv", [D, CD], F16, kind="ExternalInput")
    wot = nc.dram_tensor("wot", [CD, D], F16, kind="ExternalInput")
    mask = nc.dram_tensor("mask", [128, 128], F16, kind="ExternalInput")
    out = nc.dram_tensor("out", [S, D], F16, kind="ExternalOutput")
    dbg = os.environ.get("BASS_KERNEL_DEBUG")
    if dbg:
        dbg_q = nc.dram_tensor("dbg_q", [128, S], F16, kind="ExternalOutput")
        dbg_k = nc.dram_tensor("dbg_k", [128, S], F16, kind="ExternalOutput")
        dbg_v = nc.dram_tensor("dbg_v", [128, NSC * NH * 65], F16,
                               kind="ExternalOutput")
        dbg_ao = nc.dram_tensor("dbg_ao", [128, S], F16, kind="ExternalOutput")
        dbg_pvs = nc.dram_tensor("dbg_pvs", [64, 1024], F32,
                                 kind="ExternalOutput")
        dbg_den = nc.dram_tensor("dbg_den", [1, 1024], F32,
                                 kind="ExternalOutput")
        dbg_rcf = nc.dram_tensor("dbg_rcf", [1, 1024], F32,
                                 kind="ExternalOutput")
        dbg_pr = nc.dram_tensor("dbg_pr", [128, 1024], F16,
                                kind="ExternalOutput")

    xt_view = xt.ap().rearrange("(a p) s -> p a s", p=128)     # [128, 8, 2048]
    wqk_view = wqk.ap().rearrange("(a p) m -> p a m", p=128)   # [128, 8, 1024]
    wv_view = wv.ap().rearrange("(a p) m -> p a m", p=128)     # [128, 8, 512]
    wot_view = wot.ap().rearrange("(a p) m -> p a m", p=128)   # [128, 4, 1024]

    with TileContext(nc) as tc:
        with (
            tc.tile_pool(name="persist", bufs=1) as pers,
            tc.tile_pool(name="probs", bufs=6) as prp,
            tc.tile_pool(name="pvsb", bufs=2) as pvsp,
            tc.tile_pool(name="rcp", bufs=2) as rcp,
            tc.tile_pool(name="outp", bufs=4) as obp,
            tc.tile_pool(name="psS", bufs=2, space="PSUM") as psS,
            tc.tile_pool(name="psPV", bufs=2, space="PSUM") as psPV,
            tc.tile_pool(name="psQ", bufs=2, space="PSUM") as psQ,
        ):
            # ---- persistent SBUF tiles ----
            xt_sb = [pers.tile([128, NDT, 512], F16, tag=f"xt{i}", name=f"xt{i}")
                     for i in range(NST)]
            wqk_sb = [pers.tile([128, NDT, 128], F16, tag=f"wqk{g}", name=f"wqk{g}")
                      for g in range(8)]
            wv_sb = pers.tile([128, NDT, CD], F16, tag="wv")
            wot_sb = pers.tile([128, NPAIR, D], F16, tag="wot")
            mask_sb = pers.tile([128, 128], F16, tag="mask")
            qT = [pers.tile([128, S], F16, tag=f"qT{p}", name=f"qT{p}")
                  for p in range(NPAIR)]
            kT = [pers.tile([128, S], F16, tag=f"kT{p}", name=f"kT{p}")
                  for p in range(NPAIR)]
            aoT = [pers.tile([128, S], F16, tag=f"aoT{p}", name=f"aoT{p}")
                   for p in range(NPAIR)]
            # v natural layout with fused ones col: group (kt, head) -> 65 cols
            v65 = pers.tile([128, NSC * NH * 65], F16, tag="v65")
            v65g = v65[:].rearrange("p (g c) -> p g c", c=65)  # [128, 128, 65]

            # ---- input DMAs, ordered by first use so the upfront matmuls
            # ride the incoming stream ----
            for k in range(NDT):
                nc.sync.dma_start(wqk_sb[0][:, k, :], wqk_view[:, k, 0:128])
                nc.sync.dma_start(xt_sb[0][:, k, :], xt_view[:, k, 0:512])
            for k in range(NDT):
                nc.sync.dma_start(wqk_sb[4][:, k, :], wqk_view[:, k, 512:640])
                nc.sync.dma_start(wv_sb[:, k, :], wv_view[:, k, :])
            for i in range(1, NST):
                nc.sync.dma_start(xt_sb[i][:],
                                  xt_view[:, :, i * 512:(i + 1) * 512])
            nc.sync.dma_start(mask_sb[:], mask.ap())
            for g in (1, 5, 2, 6, 3, 7):
                nc.sync.dma_start(wqk_sb[g][:],
                                  wqk_view[:, :, g * 128:(g + 1) * 128])
            nc.sync.dma_start(wot_sb[:], wot_view)
            nc.gpsimd.load_library(library_config.proxy)

            # ones columns of every [v | 1] group
            nc.vector.memset(v65g[:, :, 64:65], 1.0)

            # ---- step generators (emitted inline or as PE filler) ----
            def qkv_group_steps(st, g):
                # q/k projection chunk: out dims g*128..(g+1)*128 (g<4: q-pair
                # g; g>=4: k-pair g-4), seq cols st*512..(st+1)*512
                ps_box = []

                def mm(k):
                    def f():
                        if k == 0:
                            ps_box.append(psQ.tile([128, 512], F32, tag="psq", name="psq"))
                        nc.tensor.matmul(
                            ps_box[0][:],
                            wqk_sb[g][:, k, :],
                            xt_sb[st][:, k, :],
                            start=(k == 0), stop=(k == NDT - 1),
                            skip_group_check=True,
                        )
                    return f

                def cp():
                    dest = qT[g] if g < 4 else kT[g - 4]
                    nc.vector.tensor_copy(
                        dest[:, st * 512:(st + 1) * 512], ps_box[0][:])

                return [mm(k) for k in range(NDT)] + [cp]

            def vpass_group_steps(t):
                # v natural for seq chunk t: out [128 seq, 512 vdims]
                st, j = t // 4, t % 4
                ps_box = []

                def mm(k):
                    def f():
                        if k == 0:
                            ps_box.append(psQ.tile([128, 512], F32, tag="psq", name="psq"))
                        nc.tensor.matmul(
                            ps_box[0][:],
                            xt_sb[st][:, k, j * 128:(j + 1) * 128],
                            wv_sb[:, k, :],
                            start=(k == 0), stop=(k == NDT - 1),
                            skip_group_check=True,
                        )
                    return f

                def cp():
                    src = ps_box[0][:].rearrange("p (h c) -> p h c", c=64)
                    nc.vector.tensor_copy(
                        v65g[:, t * NH:(t + 1) * NH, 0:64], src)

                return [mm(k) for k in range(NDT)] + [cp]

            def oproj_group_steps(t, ot):
                # O projection for seq chunk t, output cols ot*512..
                ps_box = []

                def mm(p):
                    def f():
                        if p == 0:
                            ps_box.append(psQ.tile([128, 512], F32, tag="psq", name="psq"))
                        nc.tensor.matmul(
                            ps_box[0][:],
                            aoT[p][:, t * 128:(t + 1) * 128],
                            wot_sb[:, p, ot * 512:(ot + 1) * 512],
                            start=(p == 0), stop=(p == NPAIR - 1),
                            skip_group_check=True,
                        )
                    return f

                def cp():
                    ob = obp.tile([128, 512], F16, tag="ob", name="ob")
                    nc.vector.tensor_copy(ob[:], ps_box[0][:])
                    nc.sync.dma_start(
                        out.ap()[t * 128:(t + 1) * 128,
                                 ot * 512:(ot + 1) * 512], ob[:])

                return [mm(p) for p in range(NPAIR)] + [cp]

            class StepQueue:
                def __init__(self):
                    self.q = deque()
                    self.popped = 0

                def add(self, steps):
                    self.q.extend(steps)

                def pop(self, n):
                    for _ in range(n):
                        if not self.q:
                            return
                        self.q.popleft()()
                        self.popped += 1

                def ensure(self, n):
                    while self.popped < n and self.q:
                        self.q.popleft()()
                        self.popped += 1

                def drain(self):
                    while self.q:
                        self.q.popleft()()
                        self.popped += 1

            # ---- upfront: QKV pair 0 interleaved with v-pass 0..7 ----
            for st in range(NST):
                for g in (0, 4):
                    for f in qkv_group_steps(st, g):
                        f()
                if st < 2:
                    for t in range(st * 4, st * 4 + 4):
                        for f in vpass_group_steps(t):
                            f()

            fill = StepQueue()
            for t in range(8, NSC):
                fill.add(vpass_group_steps(t))
            vpass_base = 0
            qkv_base = {0: 0}
            qkv_base[1] = 8 * 9   # after the deferred v-pass steps
            fill.add([f for st in range(NST) for g in (1, 5)
                      for f in qkv_group_steps(st, g)])

            # ---- attention: pair-outer loop with PE filler interleave ----
            for p in range(NPAIR):
                npops = 3 if p == 0 else (5 if p == 3 else 2)
                qts = (3, 2, 1, 0) if p == 3 else range(NST)
                for qt in qts:
                    if p == 0 and qt >= 2:
                        # v65 k-tiles up to 4*qt+3 must exist before this qt
                        fill.ensure(vpass_base + (4 * qt + 4 - 8) * 9)
                    if p >= 1:
                        # pair p's QKV chunks with st <= qt must be emitted
                        fill.ensure(qkv_base[p] + (qt + 1) * 2 * 9)
                    nkt = 4 * (qt + 1)
                    pv0 = psPV.tile([65, 512], F32, tag="pv")
                    pv1 = psPV.tile([65, 512], F32, tag="pv")
                    pend = None

                    def emit_pv(kt, c0, pr):
                        for h, pv in ((0, pv0), (1, pv1)):
                            gidx = kt * NH + 2 * p + h
                            nc.tensor.matmul(
                                pv[:, c0:512],
                                v65g[:, gidx, :],
                                pr[:, h * 512 + c0:(h + 1) * 512],
                                start=(kt == 0), stop=(kt == nkt - 1),
                                skip_group_check=True,
                            )

                    for kt in range(nkt):
                        o = kt * 128 - qt * 512
                        c0 = max(0, o)
                        sp = psS.tile([128, 1024], F32, tag="s")
                        pr = prp.tile([128, 1024], F16, tag="pr")
                        for h in (0, 1):
                            nc.tensor.matmul(
                                sp[:, h * 512 + c0:(h + 1) * 512],
                                kT[p][h * 64:(h + 1) * 64,
                                      kt * 128:(kt + 1) * 128],
                                qT[p][h * 64:(h + 1) * 64,
                                      qt * 512 + c0:(qt + 1) * 512],
                                start=True, stop=True,
                                tile_position=(h * 64, 0),
                            )
                        if c0 > 0:
                            spv = sp[:].rearrange("p (h c) -> p h c", c=512)
                            prv = pr[:].rearrange("p (h c) -> p h c", c=512)
                            nc.scalar.activation(prv[:, :, c0:512],
                                                 spv[:, :, c0:512],
                                                 Exp, scale=0.125)
                        else:
                            nc.scalar.activation(pr[:], sp[:], Exp, scale=0.125)
                        if o >= 0:
                            for h in (0, 1):
                                nc.vector.tensor_mul(
                                    pr[:, h * 512 + o:h * 512 + o + 128],
                                    pr[:, h * 512 + o:h * 512 + o + 128],
                                    mask_sb[:])
                        if dbg and p == 0 and qt == 0 and kt == 0:
                            nc.sync.dma_start(dbg_pr.ap(), pr[:])
                        if pend is not None:
                            emit_pv(*pend)
                        fill.pop(npops)
                        pend = (kt, c0, pr)
                    emit_pv(*pend)

                    # ---- normalization for this (pair, qt) ----
                    pvs = pvsp.tile([64, 1024], F32, tag="pvs")
                    den = rcp.tile([1, 1024], F32, tag="den")
                    nc.vector.tensor_copy(pvs[:, 0:512], pv0[0:64, :])
                    nc.vector.tensor_copy(pvs[:, 512:1024], pv1[0:64, :])
                    nc.vector.tensor_copy(den[0:1, 0:512], pv0[64:65, :])
                    nc.vector.tensor_copy(den[0:1, 512:1024], pv1[64:65, :])
                    rcf = rcp.tile([1, 1024], F32, tag="rcf")
                    nc.vector.reciprocal_approx_fast(rcf[:], den[:])
                    pbcs = rcp.tile([64, 1024], F32, tag="pbcs")
                    nc.gpsimd.partition_broadcast(pbcs[:], rcf[:])
                    q0 = qt * 512
                    nc.vector.tensor_mul(aoT[p][0:64, q0:q0 + 512],
                                         pvs[:, 0:512], pbcs[:, 0:512])
                    nc.vector.tensor_mul(aoT[p][64:128, q0:q0 + 512],
                                         pvs[:, 512:1024], pbcs[:, 512:1024])
                    if dbg and p == 0 and qt == 0:
                        nc.sync.dma_start(dbg_pvs.ap(), pvs[:])
                        nc.sync.dma_start(dbg_den.ap(), den[:])
                        nc.sync.dma_start(dbg_rcf.ap(), rcf[:])
                    fill.pop(9)
                    if p == 3:
                        # O projection for the seq chunks this qt completed
                        for t in range(qt * 4, qt * 4 + 4):
                            for ot in range(2):
                                fill.add(oproj_group_steps(t, ot))
                # stage the next-next pair's QKV as future filler
                if p < 2:
                    qkv_base[p + 2] = fill.popped + len(fill.q)
                    fill.add([f for st in range(NST) for g in (p + 2, 6 + p)
                              for f in qkv_group_steps(st, g)])
            fill.drain()
            if dbg:
                nc.sync.dma_start(dbg_q.ap(), qT[0][:])
                nc.sync.dma_start(dbg_k.ap(), kT[0][:])
                nc.sync.dma_start(dbg_v.ap(), v65[:])
                nc.sync.dma_start(dbg_ao.ap(), aoT[0][:])
    nc.compile()
    return nc


def _causal_mask():
    # mask[r, j] = 1 where key row r is visible to query column j
    r = np.arange(128)[:, None]
    j = np.arange(128)[None, :]
    return (r <= j).astype(np.float32)


def _maybe_register_ntff_hook():
    try:
        import antenv
        if getattr(antenv, "axon_hooks", None) is not None:
            return True
        import sys
        import types
        from trn_agent_boot.trn_boot import _ntff_profile_via_ctypes
        mod = types.ModuleType("antenv.axon_hooks")
        state = {"hook": _ntff_profile_via_ctypes("/opt/axon/libaxon_pjrt.so")}
        mod.set_axon_ntff_profile_hook = lambda h: state.__setitem__("hook", h)
        mod.get_axon_ntff_profile_hook = lambda: state["hook"]
        sys.modules["antenv.axon_hooks"] = mod
        antenv.axon_hooks = mod
        return True
    except Exception:
        return False


def make_in_maps(x, W_qkv, W_o):
    mask = _causal_mask().astype(np.float16)
    in_maps = []
    for c in range(N_CORES):
        b, g = c // 2, c % 2
        xt = np.ascontiguousarray(x[b].T.astype(np.float32)).astype(np.float16)
        qg = W_qkv[0 * D + g * CD:0 * D + (g + 1) * CD]
        kg = W_qkv[1 * D + g * CD:1 * D + (g + 1) * CD]
        vg = W_qkv[2 * D + g * CD:2 * D + (g + 1) * CD]
        wqk = np.ascontiguousarray(
            np.concatenate([qg, kg], axis=0).T.astype(np.float32)
        ).astype(np.float16)
        wv = np.ascontiguousarray(
            vg.T.astype(np.float32)).astype(np.float16)
        wot = np.ascontiguousarray(
            W_o[:, g * CD:(g + 1) * CD].T.astype(np.float32)
        ).astype(np.float16)
        in_maps.append({"xt": xt, "wqk": wqk, "wv": wv, "wot": wot,
                       "mask": mask})
    return in_maps


_NC_CACHE = {}


def kernel(x, W_qkv, W_o):
    assert x.shape == (B, S, D)
    in_maps = make_in_maps(x, W_qkv, W_o)

    if "nc" not in _NC_CACHE:
        _NC_CACHE["nc"] = _build_bass()
    nc = _NC_CACHE["nc"]

    trace = bool(os.environ.get("BASS_KERNEL_TRACE")) and _maybe_register_ntff_hook()
    res = run_bass_kernel_spmd(nc, in_maps, core_ids=list(range(N_CORES)),
                               trace=trace)
    if trace and res.exec_time_ns is not None:
        print(f"HW exec time: {res.exec_time_ns} ns")

    outs = []
    for b in range(B):
        acc = (res.results[2 * b]["out"].astype(np.float32)
               + res.results[2 * b + 1]["out"].astype(np.float32))
        outs.append(acc)
    return np.stack(outs, axis=0)
